# revision 13
# baseline (speedup 1.0000x reference)
"""GQA attention layer (B=2, S=2048, H=4096, 32 Q heads / 8 KV heads, HD=128)
on 8 trn2 NeuronCores.

Sharding: 2D = data-parallel over batch (2) x tensor-parallel over heads (4).
Core c -> (batch = c // 4, tp = c % 4): 8 Q heads, 2 KV heads, full sequence.
Wq/Wk/Wv split along output rows, Wo along input cols (Megatron TP); the
4 per-batch partial outputs are summed on the host (the TP unshard step).

All matmuls run in bf16 (1 cycle/col); x is streamed once per projection
phase in bf16.  Per-core phases:
  A: K/V projections from x^T (streamed), RoPE on K       -> ktr, vb (SBUF)
  B: Q projection + RoPE                                  -> qtr (SBUF)
  C: per (q-chunk, head): scores^T = K^T-tiles x Q in kt PAIRS, one exp
     (ACT, bf16) per pair, denominator via ones-matmul + AV accumulated in
     PSUM with a one-pair lag, normalize via reciprocal + K=1 broadcast
     matmul + PSUM*PSUM multiply                          -> ctx (SBUF, bf16)
  D: out = ctx^T x Wo^T (bf16, fp32 accum)                -> out (DRAM, fp32)

Modes:
  causal : skip kt tiles above the diagonal (kt > 4*qq+3); the 4 diagonal
           tiles are masked multiplicatively after exp with a precomputed
           0/1 pattern (the pattern is the same for every q-chunk).
  nomask : all 16 kt tiles, no masking.
  genmask: all 16 kt tiles, additive mask tiles streamed from DRAM
           (insurance path for non-causal non-zero masks).

RoPE runs in the natural interleaved head layout: pair (x[2i], x[2i+1])
sits at adjacent partitions, the partner is fetched with a swap-adjacent
stream_shuffle, and the sign/cos/sin tables are pre-interleaved on the host:
  rot = x * cc + shuffle(x * ss),  cc[2i]=cc[2i+1]=cos_i,
  ss[2i]=+sin_i, ss[2i+1]=-sin_i.
"""

import math

import numpy as np
import ml_dtypes

import concourse.bass as bass
import concourse.mybir as mybir
import concourse.tile as tile
from concourse import bacc
from concourse import bass_utils
from concourse.bass_interp import get_hw_module

B, S, H, NH, NKV, HD = 2, 2048, 4096, 32, 8, 128
TP = 4  # head-parallel cores per batch
N_CORES = 8
QH = NH // TP          # 8 q heads per core
KVH = NKV // TP        # 2 kv heads per core
QROWS = QH * HD        # 1024
KVROWS = KVH * HD      # 256
HT = H // 128          # 32 h (contraction) tiles
ST = S // 128          # 16 seq tiles
QCH = 512              # q-chunk width in phase C
NQQ = S // QCH
F32 = mybir.dt.float32
BF16 = mybir.dt.bfloat16
AX = mybir.AluOpType
ACTF = mybir.ActivationFunctionType
SWAP_ADJ = [i ^ 1 for i in range(32)]


def build_nc(mode: str, debug: bool = False):
    causal = mode == "causal"
    genmask = mode == "genmask"

    nc = bacc.Bacc("TRN2", target_bir_lowering=False, debug=False, num_devices=N_CORES)
    xtb = nc.dram_tensor("xtb", [H, S], BF16, kind="ExternalInput").ap()
    wqt = nc.dram_tensor("wqt", [H, QROWS], BF16, kind="ExternalInput").ap()
    wkt = nc.dram_tensor("wkt", [H, KVROWS], BF16, kind="ExternalInput").ap()
    wvt = nc.dram_tensor("wvt", [H, KVROWS], BF16, kind="ExternalInput").ap()
    wot = nc.dram_tensor("wot", [QROWS, H], BF16, kind="ExternalInput").ap()
    cs = nc.dram_tensor("cs", [128, S], BF16, kind="ExternalInput").ap()
    sc = nc.dram_tensor("sc", [128, S], BF16, kind="ExternalInput").ap()
    diagm = None
    maskt = None
    if causal:
        diagm = nc.dram_tensor("diagm", [128, 4 * QCH], BF16, kind="ExternalInput").ap()
    if genmask:
        maskt = nc.dram_tensor("maskt", [S, S], BF16, kind="ExternalInput").ap()
    out = nc.dram_tensor("out", [S, H], F32, kind="ExternalOutput").ap()
    dbg = {}
    if debug:
        dbg["k"] = nc.dram_tensor("dbg_k", [128, KVH * S], BF16, kind="ExternalOutput").ap()
        dbg["q"] = nc.dram_tensor("dbg_q", [128, QH * S], BF16, kind="ExternalOutput").ap()
        dbg["v"] = nc.dram_tensor("dbg_v", [128, KVH * ST * HD], BF16, kind="ExternalOutput").ap()
        dbg["ctx"] = nc.dram_tensor("dbg_ctx", [128, QH * S], BF16, kind="ExternalOutput").ap()

    with tile.TileContext(nc) as tc:
        with tc.tile_pool(name="persist", bufs=1) as pp:
            cs_sb = pp.tile([128, S], BF16)
            sc_sb = pp.tile([128, S], BF16)
            ktr = pp.tile([128, KVH, S], BF16)          # roped K^T (1 MB)
            vb = pp.tile([128, KVH, ST, HD], BF16)      # V [seq, hd] tiles (1 MB)
            qtr = pp.tile([128, QH, S], BF16)           # roped Q^T (4 MB)
            ctx = pp.tile([128, QH, S], BF16)           # attention output (4 MB)
            ones_bf = pp.tile([128, 1], BF16)
            ones_row = pp.tile([1, 128], BF16)
            diag_sb = None
            if causal:
                diag_sb = pp.tile([128, 4, QCH], BF16)
                nc.sync.dma_start(diag_sb[:],
                                  diagm[:].rearrange("p (t q) -> p t q", t=4))
            nc.sync.dma_start(cs_sb[:], cs[:])
            nc.sync.dma_start(sc_sb[:], sc[:])
            nc.gpsimd.memset(ones_bf[:], 1.0)
            nc.gpsimd.memset(ones_row[:], 1.0)

            # ---------------- Phase A: K/V projection + K rope ----------
            with (
                tc.tile_pool(name="wkv", bufs=1) as wkvp,
                tc.tile_pool(name="xa", bufs=8) as xap,
                tc.tile_pool(name="ropea", bufs=2) as rpa,
                tc.tile_pool(name="psk", bufs=2, space="PSUM") as psk,
                tc.tile_pool(name="psv", bufs=1, space="PSUM") as psv,
            ):
                wk_sb = wkvp.tile([128, HT, KVROWS], BF16)
                wv_sb = wkvp.tile([128, HT, KVROWS], BF16)
                for q4 in range(4):          # seq quarters of 512
                    sl = slice(q4 * 512, (q4 + 1) * 512)
                    kps = psk.tile([128, KVH, 512], F32, name="kps")
                    vps = [psv.tile([128, KVROWS], F32, name=f"vps{st}")
                           for st in range(4)]
                    for h in range(HT):
                        if q4 == 0:
                            nc.sync.dma_start(wk_sb[:, h, :],
                                              wkt[h * 128:(h + 1) * 128, :])
                            nc.sync.dma_start(wv_sb[:, h, :],
                                              wvt[h * 128:(h + 1) * 128, :])
                        xa = xap.tile([128, 512], BF16, name="xa")
                        nc.sync.dma_start(xa[:], xtb[h * 128:(h + 1) * 128, sl])
                        for r in range(KVH):
                            nc.tensor.matmul(kps[:, r, :],
                                             wk_sb[:, h, r * 128:(r + 1) * 128],
                                             xa[:],
                                             start=(h == 0), stop=(h == HT - 1))
                        for st in range(4):
                            nc.tensor.matmul(vps[st][:],
                                             xa[:, st * 128:(st + 1) * 128],
                                             wv_sb[:, h, :],
                                             start=(h == 0), stop=(h == HT - 1))
                    # rope K -> ktr: rot = x*cc + shuffle(x*ss)
                    for r in range(KVH):
                        t1 = rpa.tile([128, 512], F32, name="t1")
                        m0 = rpa.tile([128, 512], F32, name="m0")
                        sw = rpa.tile([128, 512], F32, name="sw")
                        nc.vector.tensor_tensor(t1[:], kps[:, r, :], cs_sb[:, sl], op=AX.mult)
                        nc.vector.tensor_tensor(m0[:], kps[:, r, :], sc_sb[:, sl], op=AX.mult)
                        nc.vector.stream_shuffle(sw[:], m0[:], mask=SWAP_ADJ)
                        nc.vector.tensor_tensor(ktr[:, r, sl], t1[:], sw[:], op=AX.add)
                    # evict V -> vb: vps[st][p, kv*128+d] -> vb[p, kv, q4*4+st, d]
                    for st in range(4):
                        nc.scalar.copy(
                            vb[:, :, q4 * 4 + st, :],
                            vps[st][:].rearrange("p (kv d) -> p kv d", kv=KVH))

            # ---------------- Phase B: Q projection + rope --------------
            # two half-row passes (4 heads each) to bound wq SBUF use
            with (
                tc.tile_pool(name="wq", bufs=1) as wqp,
                tc.tile_pool(name="xb", bufs=8) as xbp,
                tc.tile_pool(name="ropeb", bufs=2) as rpb,
                tc.tile_pool(name="psb", bufs=2, space="PSUM") as psb,
            ):
                for rh in range(2):
                    rsl = slice(rh * 512, (rh + 1) * 512)
                    wq_sb = wqp.tile([128, HT, 512], BF16, name="wq_sb")
                    for qc in range(4):          # seq quarters of 512
                        sl = slice(qc * 512, (qc + 1) * 512)
                        qps = psb.tile([128, 4, 512], F32, name="qps")
                        for h in range(HT):
                            if qc == 0:
                                nc.sync.dma_start(wq_sb[:, h, :],
                                                  wqt[h * 128:(h + 1) * 128, rsl])
                            xb = xbp.tile([128, 512], BF16, name="xb")
                            nc.sync.dma_start(xb[:], xtb[h * 128:(h + 1) * 128, sl])
                            for r in range(4):
                                nc.tensor.matmul(qps[:, r, :],
                                                 wq_sb[:, h, r * 128:(r + 1) * 128],
                                                 xb[:],
                                                 start=(h == 0), stop=(h == HT - 1))
                        for r in range(4):
                            head = rh * 4 + r
                            t1 = rpb.tile([128, 512], F32, name="t1")
                            m0 = rpb.tile([128, 512], F32, name="m0")
                            sw = rpb.tile([128, 512], F32, name="sw")
                            nc.vector.tensor_tensor(t1[:], qps[:, r, :], cs_sb[:, sl], op=AX.mult)
                            nc.vector.tensor_tensor(m0[:], qps[:, r, :], sc_sb[:, sl], op=AX.mult)
                            nc.vector.stream_shuffle(sw[:], m0[:], mask=SWAP_ADJ)
                            nc.vector.tensor_tensor(qtr[:, head, sl], t1[:], sw[:], op=AX.add)

            # ---------------- Phase C: attention ------------------------
            with (
                tc.tile_pool(name="expp", bufs=4) as expp,
                tc.tile_pool(name="smallc", bufs=4) as smc,
                tc.tile_pool(name="mkp", bufs=2) as mkp,
                tc.tile_pool(name="pscs", bufs=2, space="PSUM") as pscs,
                tc.tile_pool(name="pscx", bufs=2, space="PSUM") as pscx,
                tc.tile_pool(name="pscd", bufs=2, space="PSUM") as pscd,
            ):
                for qq in range(NQQ):
                    qsl = slice(qq * QCH, (qq + 1) * QCH)
                    NKT = 4 * qq + 4 if causal else ST
                    NP = NKT // 2
                    mk = None
                    if genmask:
                        mk = mkp.tile([128, ST, QCH], BF16, name="mk")
                        for kt in range(ST):
                            nc.sync.dma_start(
                                mk[:, kt, :], maskt[kt * 128:(kt + 1) * 128, qsl])
                    prev = None
                    for h in range(QH + 1):
                        if h < QH:
                            kvh = h // (QH // KVH)
                            ctxps = pscx.tile([128, QCH], F32, name="ctxps")
                            dps = pscd.tile([1, QCH], F32, name="dps")
                            eps = [None] * NP
                            for p in range(NP + 1):
                                if p < NP:
                                    # scores for kt pair (2p, 2p+1) + one exp
                                    spair = pscs.tile([128, 2, QCH], F32, name="spair")
                                    for i in range(2):
                                        kt = 2 * p + i
                                        nc.tensor.matmul(
                                            spair[:, i, :],
                                            ktr[:, kvh, kt * 128:(kt + 1) * 128],
                                            qtr[:, h, qsl],
                                            start=True, stop=True)
                                        if genmask:
                                            nc.vector.tensor_tensor(
                                                spair[:, i, :], spair[:, i, :],
                                                mk[:, kt, :], op=AX.add)
                                    ep = expp.tile([128, 2, QCH], BF16, name="ep")
                                    nc.scalar.activation(ep[:], spair[:], ACTF.Exp)
                                    if causal and p >= NP - 2:
                                        for i in range(2):
                                            t = 2 * p + i - (NKT - 4)
                                            nc.vector.tensor_tensor(
                                                ep[:, i, :], ep[:, i, :],
                                                diag_sb[:, t, :], op=AX.mult)
                                    eps[p] = ep
                                if p == 1 and h > 0:
                                    # normalize previous head: ctx = ctxps/denom
                                    p_ctxps, p_dps, hh = prev
                                    rf = smc.tile([1, QCH], F32, name="rf")
                                    nc.vector.reciprocal_approx_fast(rf[:], p_dps[:])
                                    rb = smc.tile([1, QCH], BF16, name="rb")
                                    nc.vector.tensor_copy(rb[:], rf[:])
                                    un = smc.tile([128, QCH], BF16, name="un")
                                    nc.scalar.copy(un[:], p_ctxps[:])
                                    bps = pscs.tile([128, 2, QCH], F32, name="spair")
                                    nc.tensor.matmul(bps[:, 0, :], ones_row[:], rb[:],
                                                     start=True, stop=True)
                                    nc.vector.tensor_tensor(
                                        ctx[:, hh, qsl], un[:], bps[:, 0, :],
                                        op=AX.mult)
                                if p >= 1:
                                    # denominator + AV for pair p-1 (one-pair lag)
                                    ep = eps[p - 1]
                                    for i in range(2):
                                        kt = 2 * (p - 1) + i
                                        nc.tensor.matmul(
                                            dps[:], ones_bf[:], ep[:, i, :],
                                            start=(kt == 0), stop=(kt == NKT - 1))
                                        nc.tensor.matmul(
                                            ctxps[:], vb[:, kvh, kt, :], ep[:, i, :],
                                            start=(kt == 0), stop=(kt == NKT - 1))
                            prev = (ctxps, dps, h)
                    # last head of the chunk
                    p_ctxps, p_dps, hh = prev
                    rf = smc.tile([1, QCH], F32, name="rf")
                    nc.vector.reciprocal_approx_fast(rf[:], p_dps[:])
                    rb = smc.tile([1, QCH], BF16, name="rb")
                    nc.vector.tensor_copy(rb[:], rf[:])
                    un = smc.tile([128, QCH], BF16, name="un")
                    nc.scalar.copy(un[:], p_ctxps[:])
                    bps = pscs.tile([128, 2, QCH], F32, name="spair")
                    nc.tensor.matmul(bps[:, 0, :], ones_row[:], rb[:],
                                     start=True, stop=True)
                    nc.vector.tensor_tensor(
                        ctx[:, hh, qsl], un[:], bps[:, 0, :], op=AX.mult)

            if debug:
                nc.sync.dma_start(dbg["k"][:], ktr[:].rearrange("p kv s -> p (kv s)"))
                nc.sync.dma_start(dbg["v"][:], vb[:].rearrange("p kv st d -> p (kv st d)"))
                nc.sync.dma_start(dbg["q"][:], qtr[:].rearrange("p h s -> p (h s)"))
                nc.sync.dma_start(dbg["ctx"][:], ctx[:].rearrange("p h s -> p (h s)"))

            # ---------------- Phase D: output projection ------------
            with (
                tc.tile_pool(name="wo", bufs=1) as wop,
                tc.tile_pool(name="ob", bufs=3) as obp,
                tc.tile_pool(name="psd", bufs=2, space="PSUM") as psd,
            ):
                wo_sb = wop.tile([128, QH, H], BF16)
                for h in range(QH):
                    nc.sync.dma_start(wo_sb[:, h, :], wot[h * 128:(h + 1) * 128, :])
                for st in range(ST):
                    for half in range(2):
                        ops = psd.tile([128, 2048], F32, name="ops")
                        for h in range(QH):
                            for n in range(4):
                                nc.tensor.matmul(
                                    ops[:, n * 512:(n + 1) * 512],
                                    ctx[:, h, st * 128:(st + 1) * 128],
                                    wo_sb[:, h, half * 2048 + n * 512:half * 2048 + (n + 1) * 512],
                                    start=(h == 0), stop=(h == QH - 1))
                        osb = obp.tile([128, 2048], F32, name="osb")
                        nc.scalar.copy(osb[:], ops[:])
                        nc.sync.dma_start(
                            out[st * 128:(st + 1) * 128,
                                half * 2048:(half + 1) * 2048],
                            osb[:])

    nc.compile()
    nc.m = get_hw_module(nc.m)
    return nc


_NC_CACHE = {}


def _get_nc(mode: str, debug: bool = False):
    key = (mode, debug)
    if key not in _NC_CACHE:
        _NC_CACHE[key] = build_nc(mode, debug)
    return _NC_CACHE[key]


def _detect_mode(attention_mask):
    if not np.any(attention_mask):
        return "nomask"
    tril = np.tril(np.ones((S, S), dtype=bool))
    for b in range(attention_mask.shape[0]):
        m = attention_mask[b, 0]
        if not (np.all(m[tril] == 0.0) and np.all(m[~tril] < -1e30)):
            return "genmask"
    return "causal"


def _build_diag_pattern():
    # diag tile t: rows k (partition), cols q in [0, QCH);
    # unmasked (1.0) iff t*128 + k <= q
    k = np.arange(128)[:, None]
    q = np.arange(QCH)[None, :]
    tiles = [(t * 128 + k <= q).astype(np.float32) for t in range(4)]
    return np.concatenate(tiles, axis=1).astype(ml_dtypes.bfloat16)


def kernel(hidden_states, cos, sin, position_ids, attention_mask, Wq, Wk, Wv, Wo,
           _trace=False, _debug=False):
    hidden_states = np.asarray(hidden_states, np.float32)
    cos = np.asarray(cos, np.float32)
    sin = np.asarray(sin, np.float32)
    position_ids = np.asarray(position_ids)
    attention_mask = np.asarray(attention_mask, np.float32)
    Wq = np.asarray(Wq, np.float32)
    Wk = np.asarray(Wk, np.float32)
    Wv = np.asarray(Wv, np.float32)
    Wo = np.asarray(Wo, np.float32)

    mode = _detect_mode(attention_mask)
    nc = _get_nc(mode, _debug)

    scale = 1.0 / math.sqrt(HD)
    wqt_full = np.ascontiguousarray((Wq * scale).T).astype(ml_dtypes.bfloat16)
    wkt_full = np.ascontiguousarray(Wk.T).astype(ml_dtypes.bfloat16)
    wvt_full = np.ascontiguousarray(Wv.T).astype(ml_dtypes.bfloat16)
    wot_full = np.ascontiguousarray(Wo.T).astype(ml_dtypes.bfloat16)

    pos = np.asarray(position_ids, np.int64)
    diag = _build_diag_pattern() if mode == "causal" else None
    per_batch = {}
    for b in range(B):
        xtb = np.ascontiguousarray(hidden_states[b].T).astype(ml_dtypes.bfloat16)
        cg = cos[pos[b]]                                     # [2048, 64]
        sg = sin[pos[b]]
        cs_b = np.repeat(cg.T, 2, axis=0).astype(ml_dtypes.bfloat16)   # cc
        sc_b = np.empty((HD, S), np.float32)                           # ss
        sc_b[0::2] = sg.T
        sc_b[1::2] = -sg.T
        sc_b = sc_b.astype(ml_dtypes.bfloat16)
        mt_b = None
        if mode == "genmask":
            mt_b = np.ascontiguousarray(attention_mask[b, 0].T).astype(ml_dtypes.bfloat16)
        per_batch[b] = (xtb, cs_b, sc_b, mt_b)

    in_maps = []
    for c in range(N_CORES):
        b, tp = c // TP, c % TP
        xtb, cs_b, sc_b, mt_b = per_batch[b]
        m = {
            "xtb": xtb,
            "wqt": np.ascontiguousarray(wqt_full[:, tp * QROWS:(tp + 1) * QROWS]),
            "wkt": np.ascontiguousarray(wkt_full[:, tp * KVROWS:(tp + 1) * KVROWS]),
            "wvt": np.ascontiguousarray(wvt_full[:, tp * KVROWS:(tp + 1) * KVROWS]),
            "wot": np.ascontiguousarray(wot_full[tp * QROWS:(tp + 1) * QROWS, :]),
            "cs": cs_b,
            "sc": sc_b,
        }
        if mode == "causal":
            m["diagm"] = diag
        if mode == "genmask":
            m["maskt"] = mt_b
        in_maps.append(m)

    res = bass_utils.run_bass_kernel_spmd(
        nc, in_maps, core_ids=list(range(N_CORES)), trace=_trace)

    out = np.zeros((B, S, H), np.float32)
    for c in range(N_CORES):
        out[c // TP] += res.results[c]["out"]
    if _trace:
        kernel._last_results = res
    return out


# revision 14
# speedup vs baseline: 1.3603x; 1.3603x over previous
"""GQA attention layer (B=2, S=2048, H=4096, 32 Q heads / 8 KV heads, HD=128)
on 8 trn2 NeuronCores.

Sharding: 2D = data-parallel over batch (2) x tensor-parallel over heads (4).
Core c -> (batch = c // 4, tp = c % 4): 8 Q heads, 2 KV heads, full sequence.
Wq/Wk/Wv split along output rows, Wo along input cols (Megatron TP); the
4 per-batch partial outputs are summed on the host (the TP unshard step).

All matmuls run in bf16 (1 cycle/col); x is streamed once per projection
phase in bf16.  Per-core phases:
  A: K/V projections from x^T (streamed), RoPE on K       -> ktr, vb (SBUF)
  B: Q projection + RoPE                                  -> qtr (SBUF)
  C: per (q-chunk, head): scores^T = K^T-tiles x Q in kt PAIRS, one exp
     (ACT, bf16) per pair, denominator via ones-matmul + AV accumulated in
     PSUM with a one-pair lag, normalize via reciprocal + K=1 broadcast
     matmul + PSUM*PSUM multiply                          -> ctx (SBUF, bf16)
  D: out = ctx^T x Wo^T (bf16, fp32 accum)                -> out (DRAM, fp32)

Modes:
  causal : skip kt tiles above the diagonal (kt > 4*qq+3); the 4 diagonal
           tiles are masked multiplicatively after exp with a precomputed
           0/1 pattern (the pattern is the same for every q-chunk).
  nomask : all 16 kt tiles, no masking.
  genmask: all 16 kt tiles, additive mask tiles streamed from DRAM
           (insurance path for non-causal non-zero masks).

RoPE runs in the natural interleaved head layout: pair (x[2i], x[2i+1])
sits at adjacent partitions, the partner is fetched with a swap-adjacent
stream_shuffle, and the sign/cos/sin tables are pre-interleaved on the host:
  rot = x * cc + shuffle(x * ss),  cc[2i]=cc[2i+1]=cos_i,
  ss[2i]=+sin_i, ss[2i+1]=-sin_i.
"""

import math

import numpy as np
import ml_dtypes

import concourse.bass as bass
import concourse.mybir as mybir
import concourse.tile as tile
from concourse import bacc
from concourse import bass_utils
from concourse.bass_interp import get_hw_module

B, S, H, NH, NKV, HD = 2, 2048, 4096, 32, 8, 128
TP = 4  # head-parallel cores per batch
N_CORES = 8
QH = NH // TP          # 8 q heads per core
KVH = NKV // TP        # 2 kv heads per core
QROWS = QH * HD        # 1024
KVROWS = KVH * HD      # 256
HT = H // 128          # 32 h (contraction) tiles
ST = S // 128          # 16 seq tiles
QCH = 512              # q-chunk width in phase C
NQQ = S // QCH
F32 = mybir.dt.float32
BF16 = mybir.dt.bfloat16
AX = mybir.AluOpType
ACTF = mybir.ActivationFunctionType
SWAP_ADJ = [i ^ 1 for i in range(32)]


def build_nc(mode: str, debug: bool = False):
    causal = mode == "causal"
    genmask = mode == "genmask"

    nc = bacc.Bacc("TRN2", target_bir_lowering=False, debug=False, num_devices=N_CORES)
    xtb = nc.dram_tensor("xtb", [H, S], BF16, kind="ExternalInput").ap()
    wqt = nc.dram_tensor("wqt", [H, QROWS], BF16, kind="ExternalInput").ap()
    wkt = nc.dram_tensor("wkt", [H, KVROWS], BF16, kind="ExternalInput").ap()
    wvt = nc.dram_tensor("wvt", [H, KVROWS], BF16, kind="ExternalInput").ap()
    wot = nc.dram_tensor("wot", [QROWS, H], BF16, kind="ExternalInput").ap()
    cs = nc.dram_tensor("cs", [128, S], BF16, kind="ExternalInput").ap()
    sc = nc.dram_tensor("sc", [128, S], BF16, kind="ExternalInput").ap()
    diagm = None
    maskt = None
    if causal:
        diagm = nc.dram_tensor("diagm", [128, 4 * QCH], BF16, kind="ExternalInput").ap()
    if genmask:
        maskt = nc.dram_tensor("maskt", [S, S], BF16, kind="ExternalInput").ap()
    out = nc.dram_tensor("out", [S, H], F32, kind="ExternalOutput").ap()
    dbg = {}
    if debug:
        dbg["k"] = nc.dram_tensor("dbg_k", [128, KVH * S], BF16, kind="ExternalOutput").ap()
        dbg["q"] = nc.dram_tensor("dbg_q", [128, QH * S], BF16, kind="ExternalOutput").ap()
        dbg["v"] = nc.dram_tensor("dbg_v", [128, KVH * ST * HD], BF16, kind="ExternalOutput").ap()
        dbg["ctx"] = nc.dram_tensor("dbg_ctx", [128, QH * S], BF16, kind="ExternalOutput").ap()

    with tile.TileContext(nc) as tc:
        with tc.tile_pool(name="persist", bufs=1) as pp:
            cs_sb = pp.tile([128, S], BF16)
            sc_sb = pp.tile([128, S], BF16)
            ktr = pp.tile([128, KVH, S], BF16)          # roped K^T (1 MB)
            vb = pp.tile([128, KVH, ST, HD], BF16)      # V [seq, hd] tiles (1 MB)
            qtr = pp.tile([128, QH, S], BF16)           # roped Q^T (4 MB)
            ctx = pp.tile([128, QH, S], BF16)           # attention output (4 MB)
            ones_bf = pp.tile([128, 1], BF16)
            ones_row = pp.tile([1, 128], BF16)
            diag_sb = None
            if causal:
                diag_sb = pp.tile([128, 4, QCH], BF16)
                nc.sync.dma_start(diag_sb[:],
                                  diagm[:].rearrange("p (t q) -> p t q", t=4))
            nc.sync.dma_start(cs_sb[:], cs[:])
            nc.sync.dma_start(sc_sb[:], sc[:])
            nc.gpsimd.memset(ones_bf[:], 1.0)
            nc.gpsimd.memset(ones_row[:], 1.0)

            # ---------------- Phase A: K/V projection + K rope ----------
            with (
                tc.tile_pool(name="wkv", bufs=1) as wkvp,
                tc.tile_pool(name="xa", bufs=8) as xap,
                tc.tile_pool(name="ropea", bufs=2) as rpa,
                tc.tile_pool(name="psk", bufs=2, space="PSUM") as psk,
                tc.tile_pool(name="psv", bufs=1, space="PSUM") as psv,
            ):
                wk_sb = wkvp.tile([128, HT, KVROWS], BF16)
                wv_sb = wkvp.tile([128, HT, KVROWS], BF16)
                for q4 in range(4):          # seq quarters of 512
                    sl = slice(q4 * 512, (q4 + 1) * 512)
                    kps = psk.tile([128, KVH, 512], F32, name="kps")
                    vps = [psv.tile([128, KVROWS], F32, name=f"vps{st}")
                           for st in range(4)]
                    for h in range(HT):
                        if q4 == 0:
                            nc.sync.dma_start(wk_sb[:, h, :],
                                              wkt[h * 128:(h + 1) * 128, :])
                            nc.sync.dma_start(wv_sb[:, h, :],
                                              wvt[h * 128:(h + 1) * 128, :])
                        xa = xap.tile([128, 512], BF16, name="xa")
                        nc.sync.dma_start(xa[:], xtb[h * 128:(h + 1) * 128, sl])
                        for r in range(KVH):
                            nc.tensor.matmul(kps[:, r, :],
                                             wk_sb[:, h, r * 128:(r + 1) * 128],
                                             xa[:],
                                             start=(h == 0), stop=(h == HT - 1))
                        for st in range(4):
                            nc.tensor.matmul(vps[st][:],
                                             xa[:, st * 128:(st + 1) * 128],
                                             wv_sb[:, h, :],
                                             start=(h == 0), stop=(h == HT - 1))
                    # rope K -> ktr: rot = x*cc + shuffle(x*ss)
                    for r in range(KVH):
                        t1 = rpa.tile([128, 512], F32, name="t1")
                        m0 = rpa.tile([128, 512], F32, name="m0")
                        sw = rpa.tile([128, 512], F32, name="sw")
                        nc.vector.tensor_tensor(t1[:], kps[:, r, :], cs_sb[:, sl], op=AX.mult)
                        nc.vector.tensor_tensor(m0[:], kps[:, r, :], sc_sb[:, sl], op=AX.mult)
                        nc.vector.stream_shuffle(sw[:], m0[:], mask=SWAP_ADJ)
                        nc.vector.tensor_tensor(ktr[:, r, sl], t1[:], sw[:], op=AX.add)
                    # evict V -> vb: vps[st][p, kv*128+d] -> vb[p, kv, q4*4+st, d]
                    for st in range(4):
                        nc.scalar.copy(
                            vb[:, :, q4 * 4 + st, :],
                            vps[st][:].rearrange("p (kv d) -> p kv d", kv=KVH))

            # ---------------- Phase B: Q projection + rope --------------
            # two half-row passes (4 heads each) to bound wq SBUF use
            with (
                tc.tile_pool(name="wq", bufs=1) as wqp,
                tc.tile_pool(name="xb", bufs=8) as xbp,
                tc.tile_pool(name="ropeb", bufs=2) as rpb,
                tc.tile_pool(name="psb", bufs=2, space="PSUM") as psb,
            ):
                for rh in range(2):
                    rsl = slice(rh * 512, (rh + 1) * 512)
                    wq_sb = wqp.tile([128, HT, 512], BF16, name="wq_sb")
                    for qc in range(4):          # seq quarters of 512
                        sl = slice(qc * 512, (qc + 1) * 512)
                        qps = psb.tile([128, 4, 512], F32, name="qps")
                        for h in range(HT):
                            if qc == 0:
                                nc.sync.dma_start(wq_sb[:, h, :],
                                                  wqt[h * 128:(h + 1) * 128, rsl])
                            xb = xbp.tile([128, 512], BF16, name="xb")
                            nc.sync.dma_start(xb[:], xtb[h * 128:(h + 1) * 128, sl])
                            for r in range(4):
                                nc.tensor.matmul(qps[:, r, :],
                                                 wq_sb[:, h, r * 128:(r + 1) * 128],
                                                 xb[:],
                                                 start=(h == 0), stop=(h == HT - 1))
                        for r in range(4):
                            head = rh * 4 + r
                            t1 = rpb.tile([128, 512], F32, name="t1")
                            m0 = rpb.tile([128, 512], F32, name="m0")
                            sw = rpb.tile([128, 512], F32, name="sw")
                            nc.vector.tensor_tensor(t1[:], qps[:, r, :], cs_sb[:, sl], op=AX.mult)
                            nc.vector.tensor_tensor(m0[:], qps[:, r, :], sc_sb[:, sl], op=AX.mult)
                            nc.vector.stream_shuffle(sw[:], m0[:], mask=SWAP_ADJ)
                            nc.vector.tensor_tensor(qtr[:, head, sl], t1[:], sw[:], op=AX.add)

            # ---------------- Phase C: attention ------------------------
            # Heads processed in pairs sharing the kv head: the two scores
            # (and AV) matmuls per kt share the stationary operand, which
            # the PE runs back-to-back without a weight-reload bubble.  One
            # exp per kt covers both heads.  The denominator uses kt-pair
            # sums computed on the (otherwise idle) GpSimd engine, halving
            # the ones-matmul count.  AV lags exp by 2 kt, the denominator
            # by 3, so the PE never waits on ACT/GpSimd.
            def normalize(p_ctxps, p_dps, heads, qsl):
                for i in range(2):
                    rf = smc.tile([1, QCH], F32, name="rf")
                    nc.vector.reciprocal_approx_fast(rf[:], p_dps[i][:])
                    rb = smc.tile([1, QCH], BF16, name="rb")
                    nc.vector.tensor_copy(rb[:], rf[:])
                    un = smc.tile([128, QCH], BF16, name="un")
                    nc.vector.tensor_copy(un[:], p_ctxps[i][:])
                    bps = pscs.tile([128, 2, QCH], F32, name="sp")
                    nc.tensor.matmul(bps[:, 0, :], ones_row[:], rb[:],
                                     start=True, stop=True)
                    nc.vector.tensor_tensor(
                        ctx[:, heads[i], qsl], un[:], bps[:, 0, :], op=AX.mult)

            with (
                tc.tile_pool(name="expp", bufs=4) as expp,
                tc.tile_pool(name="sump", bufs=2) as sump,
                tc.tile_pool(name="smallc", bufs=4) as smc,
                tc.tile_pool(name="mkp", bufs=2) as mkp,
                tc.tile_pool(name="pscs", bufs=2, space="PSUM") as pscs,
                tc.tile_pool(name="pscx", bufs=2, space="PSUM") as pscx,
                tc.tile_pool(name="pscd", bufs=2, space="PSUM") as pscd,
            ):
                prev = None
                for qq in range(NQQ):
                    qsl = slice(qq * QCH, (qq + 1) * QCH)
                    NKT = 4 * qq + 4 if causal else ST
                    mk = None
                    if genmask:
                        mk = mkp.tile([128, ST, QCH], BF16, name="mk")
                        for kt in range(ST):
                            nc.sync.dma_start(
                                mk[:, kt, :], maskt[kt * 128:(kt + 1) * 128, qsl])
                    for hp in range(QH // 2):
                        kvh = hp // (QH // KVH // 2)
                        heads = (2 * hp, 2 * hp + 1)
                        ctxps = [pscx.tile([128, QCH], F32, name="ctxps")
                                 for _ in range(2)]
                        dps = [pscd.tile([1, QCH], F32, name="dps")
                               for _ in range(2)]
                        eps = [None] * NKT
                        esums = [None] * (NKT // 2)
                        for t in range(NKT + 3):
                            if t < NKT:
                                sp = pscs.tile([128, 2, QCH], F32, name="sp")
                                for i in range(2):
                                    nc.tensor.matmul(
                                        sp[:, i, :],
                                        ktr[:, kvh, t * 128:(t + 1) * 128],
                                        qtr[:, heads[i], qsl],
                                        start=True, stop=True)
                                    if genmask:
                                        nc.vector.tensor_tensor(
                                            sp[:, i, :], sp[:, i, :],
                                            mk[:, t, :], op=AX.add)
                                ep = expp.tile([128, 2, QCH], BF16, name="ep")
                                nc.scalar.activation(ep[:], sp[:], ACTF.Exp)
                                if causal and t >= NKT - 4:
                                    d = t - (NKT - 4)
                                    for i in range(2):
                                        nc.vector.tensor_tensor(
                                            ep[:, i, :], ep[:, i, :],
                                            diag_sb[:, d, :], op=AX.mult)
                                eps[t] = ep
                                if t % 2 == 1:
                                    es = sump.tile([128, 2, QCH], BF16, name="es")
                                    for i in range(2):
                                        nc.gpsimd.tensor_tensor(
                                            es[:, i, :], eps[t - 1][:, i, :],
                                            eps[t][:, i, :], op=AX.add)
                                    esums[t // 2] = es
                            if t == 1 and prev is not None:
                                normalize(*prev)
                                prev = None
                            kt = t - 2
                            if 0 <= kt < NKT:
                                ep = eps[kt]
                                for i in range(2):
                                    nc.tensor.matmul(
                                        ctxps[i][:], vb[:, kvh, kt, :], ep[:, i, :],
                                        start=(kt == 0), stop=(kt == NKT - 1))
                            ktd = t - 3
                            if 0 <= ktd < NKT and ktd % 2 == 1:
                                pi = ktd // 2
                                es = esums[pi]
                                for i in range(2):
                                    nc.tensor.matmul(
                                        dps[i][:], ones_bf[:], es[:, i, :],
                                        start=(pi == 0), stop=(pi == NKT // 2 - 1))
                        prev = (ctxps, dps, heads, qsl)
                if prev is not None:
                    normalize(*prev)

            if debug:
                nc.sync.dma_start(dbg["k"][:], ktr[:].rearrange("p kv s -> p (kv s)"))
                nc.sync.dma_start(dbg["v"][:], vb[:].rearrange("p kv st d -> p (kv st d)"))
                nc.sync.dma_start(dbg["q"][:], qtr[:].rearrange("p h s -> p (h s)"))
                nc.sync.dma_start(dbg["ctx"][:], ctx[:].rearrange("p h s -> p (h s)"))

            # ---------------- Phase D: output projection ------------
            with (
                tc.tile_pool(name="wo", bufs=1) as wop,
                tc.tile_pool(name="ob", bufs=3) as obp,
                tc.tile_pool(name="psd", bufs=2, space="PSUM") as psd,
            ):
                wo_sb = wop.tile([128, QH, H], BF16)
                for h in range(QH):
                    nc.sync.dma_start(wo_sb[:, h, :], wot[h * 128:(h + 1) * 128, :])
                for st in range(ST):
                    for half in range(2):
                        ops = psd.tile([128, 2048], F32, name="ops")
                        for h in range(QH):
                            for n in range(4):
                                nc.tensor.matmul(
                                    ops[:, n * 512:(n + 1) * 512],
                                    ctx[:, h, st * 128:(st + 1) * 128],
                                    wo_sb[:, h, half * 2048 + n * 512:half * 2048 + (n + 1) * 512],
                                    start=(h == 0), stop=(h == QH - 1))
                        osb = obp.tile([128, 2048], F32, name="osb")
                        nc.scalar.copy(osb[:], ops[:])
                        nc.sync.dma_start(
                            out[st * 128:(st + 1) * 128,
                                half * 2048:(half + 1) * 2048],
                            osb[:])

    nc.compile()
    nc.m = get_hw_module(nc.m)
    return nc


_NC_CACHE = {}


def _get_nc(mode: str, debug: bool = False):
    key = (mode, debug)
    if key not in _NC_CACHE:
        _NC_CACHE[key] = build_nc(mode, debug)
    return _NC_CACHE[key]


def _detect_mode(attention_mask):
    if not np.any(attention_mask):
        return "nomask"
    tril = np.tril(np.ones((S, S), dtype=bool))
    for b in range(attention_mask.shape[0]):
        m = attention_mask[b, 0]
        if not (np.all(m[tril] == 0.0) and np.all(m[~tril] < -1e30)):
            return "genmask"
    return "causal"


def _build_diag_pattern():
    # diag tile t: rows k (partition), cols q in [0, QCH);
    # unmasked (1.0) iff t*128 + k <= q
    k = np.arange(128)[:, None]
    q = np.arange(QCH)[None, :]
    tiles = [(t * 128 + k <= q).astype(np.float32) for t in range(4)]
    return np.concatenate(tiles, axis=1).astype(ml_dtypes.bfloat16)


def kernel(hidden_states, cos, sin, position_ids, attention_mask, Wq, Wk, Wv, Wo,
           _trace=False, _debug=False):
    hidden_states = np.asarray(hidden_states, np.float32)
    cos = np.asarray(cos, np.float32)
    sin = np.asarray(sin, np.float32)
    position_ids = np.asarray(position_ids)
    attention_mask = np.asarray(attention_mask, np.float32)
    Wq = np.asarray(Wq, np.float32)
    Wk = np.asarray(Wk, np.float32)
    Wv = np.asarray(Wv, np.float32)
    Wo = np.asarray(Wo, np.float32)

    mode = _detect_mode(attention_mask)
    nc = _get_nc(mode, _debug)

    scale = 1.0 / math.sqrt(HD)
    wqt_full = np.ascontiguousarray((Wq * scale).T).astype(ml_dtypes.bfloat16)
    wkt_full = np.ascontiguousarray(Wk.T).astype(ml_dtypes.bfloat16)
    wvt_full = np.ascontiguousarray(Wv.T).astype(ml_dtypes.bfloat16)
    wot_full = np.ascontiguousarray(Wo.T).astype(ml_dtypes.bfloat16)

    pos = np.asarray(position_ids, np.int64)
    diag = _build_diag_pattern() if mode == "causal" else None
    per_batch = {}
    for b in range(B):
        xtb = np.ascontiguousarray(hidden_states[b].T).astype(ml_dtypes.bfloat16)
        cg = cos[pos[b]]                                     # [2048, 64]
        sg = sin[pos[b]]
        cs_b = np.repeat(cg.T, 2, axis=0).astype(ml_dtypes.bfloat16)   # cc
        sc_b = np.empty((HD, S), np.float32)                           # ss
        sc_b[0::2] = sg.T
        sc_b[1::2] = -sg.T
        sc_b = sc_b.astype(ml_dtypes.bfloat16)
        mt_b = None
        if mode == "genmask":
            mt_b = np.ascontiguousarray(attention_mask[b, 0].T).astype(ml_dtypes.bfloat16)
        per_batch[b] = (xtb, cs_b, sc_b, mt_b)

    in_maps = []
    for c in range(N_CORES):
        b, tp = c // TP, c % TP
        xtb, cs_b, sc_b, mt_b = per_batch[b]
        m = {
            "xtb": xtb,
            "wqt": np.ascontiguousarray(wqt_full[:, tp * QROWS:(tp + 1) * QROWS]),
            "wkt": np.ascontiguousarray(wkt_full[:, tp * KVROWS:(tp + 1) * KVROWS]),
            "wvt": np.ascontiguousarray(wvt_full[:, tp * KVROWS:(tp + 1) * KVROWS]),
            "wot": np.ascontiguousarray(wot_full[tp * QROWS:(tp + 1) * QROWS, :]),
            "cs": cs_b,
            "sc": sc_b,
        }
        if mode == "causal":
            m["diagm"] = diag
        if mode == "genmask":
            m["maskt"] = mt_b
        in_maps.append(m)

    res = bass_utils.run_bass_kernel_spmd(
        nc, in_maps, core_ids=list(range(N_CORES)), trace=_trace)

    out = np.zeros((B, S, H), np.float32)
    for c in range(N_CORES):
        out[c // TP] += res.results[c]["out"]
    if _trace:
        kernel._last_results = res
    return out


# revision 21
# speedup vs baseline: 1.3821x; 1.0160x over previous
"""GQA attention layer (B=2, S=2048, H=4096, 32 Q heads / 8 KV heads, HD=128)
on 8 trn2 NeuronCores.

Sharding: 2D = data-parallel over batch (2) x tensor-parallel over heads (4).
Core c -> (batch = c // 4, tp = c % 4): 8 Q heads, 2 KV heads, full sequence.
Wq/Wk/Wv split along output rows, Wo along input cols (Megatron TP); the
4 per-batch partial outputs are summed on the host (the TP unshard step).

All matmuls run in bf16 (1 cycle/col).  x^T is streamed once into SBUF
(16 MB) during phase A and stays resident through phase B, so the Q
projection can reuse each stationary weight tile across 4 moving seq
chunks (the PE runs same-stationary matmuls back-to-back without a
weight-reload bubble).  Per-core phases:
  A: K/V projections (streams x into SBUF), RoPE on K     -> ktr, vb (SBUF)
  B: Q projection + RoPE                                  -> qtr (DRAM)
  C: attention per (q-chunk, head-pair), see below        -> ctx (SBUF, bf16)
  D: out = ctx^T x Wo^T (bf16, fp32 accum)                -> out (DRAM, fp32)

Phase C processes heads in pairs sharing the kv head: the two scores (and
AV) matmuls per kt share the stationary operand; one exp (ACT) per kt
covers both heads.  The denominator uses kt-pair sums computed on the
otherwise-idle GpSimd engine, halving the ones-matmul count.  AV lags exp
by 3 slots and the denominator by 5 so the PE never waits on ACT/GpSimd.

Modes:
  causal : skip kt tiles above the diagonal (kt > 4*qq+3); the 4 diagonal
           tiles are masked multiplicatively after exp with a precomputed
           0/1 pattern (the same pattern for every q-chunk), and are
           processed first so the DVE masking stays off the steady-state
           exp->AV path.
  nomask : all 16 kt tiles, no masking.
  genmask: all 16 kt tiles, additive mask tiles streamed from DRAM
           (insurance path for non-causal non-zero masks).

RoPE runs in the natural interleaved head layout: pair (x[2i], x[2i+1])
sits at adjacent partitions, the partner is fetched with a swap-adjacent
stream_shuffle, and the sign/cos/sin tables are pre-interleaved on the host:
  rot = x * cc + shuffle(x * ss),  cc[2i]=cc[2i+1]=cos_i,
  ss[2i]=+sin_i, ss[2i+1]=-sin_i.
"""

import math

import numpy as np
import ml_dtypes

import concourse.bass as bass
import concourse.mybir as mybir
import concourse.tile as tile
from concourse import bacc
from concourse import bass_utils
from concourse.bass_interp import get_hw_module

B, S, H, NH, NKV, HD = 2, 2048, 4096, 32, 8, 128
TP = 4  # head-parallel cores per batch
N_CORES = 8
QH = NH // TP          # 8 q heads per core
KVH = NKV // TP        # 2 kv heads per core
QROWS = QH * HD        # 1024
KVROWS = KVH * HD      # 256
HT = H // 128          # 32 h (contraction) tiles
ST = S // 128          # 16 seq tiles
QCH = 512              # q-chunk width in phase C
NQQ = S // QCH
F32 = mybir.dt.float32
BF16 = mybir.dt.bfloat16
AX = mybir.AluOpType
ACTF = mybir.ActivationFunctionType
SWAP_ADJ = [i ^ 1 for i in range(32)]


def build_nc(mode: str, debug: bool = False):
    causal = mode == "causal"
    genmask = mode == "genmask"

    nc = bacc.Bacc("TRN2", target_bir_lowering=False, debug=False, num_devices=N_CORES)
    xtb = nc.dram_tensor("xtb", [H, S], BF16, kind="ExternalInput").ap()
    wqt = nc.dram_tensor("wqt", [H, QROWS], BF16, kind="ExternalInput").ap()
    wkt = nc.dram_tensor("wkt", [H, KVROWS], BF16, kind="ExternalInput").ap()
    wvt = nc.dram_tensor("wvt", [H, KVROWS], BF16, kind="ExternalInput").ap()
    wot = nc.dram_tensor("wot", [QROWS, H], BF16, kind="ExternalInput").ap()
    cs = nc.dram_tensor("cs", [128, S], BF16, kind="ExternalInput").ap()
    sc = nc.dram_tensor("sc", [128, S], BF16, kind="ExternalInput").ap()
    diagm = None
    maskt = None
    if causal:
        diagm = nc.dram_tensor("diagm", [128, 4 * QCH], BF16, kind="ExternalInput").ap()
    if genmask:
        maskt = nc.dram_tensor("maskt", [S, S], BF16, kind="ExternalInput").ap()
    out = nc.dram_tensor("out", [S, H], F32, kind="ExternalOutput").ap()
    dbg = {}
    if debug:
        dbg["k"] = nc.dram_tensor("dbg_k", [128, KVH * S], BF16, kind="ExternalOutput").ap()
        dbg["v"] = nc.dram_tensor("dbg_v", [128, KVH * ST * HD], BF16, kind="ExternalOutput").ap()
        dbg["ctx"] = nc.dram_tensor("dbg_ctx", [128, QH * S], BF16, kind="ExternalOutput").ap()

    with tile.TileContext(nc) as tc:
        with (
            tc.tile_pool(name="persist", bufs=1) as pp,
            tc.tile_pool(name="dram", bufs=1, space="DRAM") as dpool,
        ):
            cs_sb = pp.tile([128, S], BF16)
            sc_sb = pp.tile([128, S], BF16)
            ktr = pp.tile([128, KVH, S], BF16)          # roped K^T (1 MB)
            vb = pp.tile([128, KVH, ST, HD], BF16)      # V [seq, hd] tiles (1 MB)
            ones_bf = pp.tile([128, 1], BF16)
            ones_row = pp.tile([1, 128], BF16)
            diag_sb = None
            if causal:
                diag_sb = pp.tile([128, 4, QCH], BF16)
                nc.sync.dma_start(diag_sb[:],
                                  diagm[:].rearrange("p (t q) -> p t q", t=4))
            nc.sync.dma_start(cs_sb[:], cs[:])
            nc.sync.dma_start(sc_sb[:], sc[:])
            nc.gpsimd.memset(ones_bf[:], 1.0)
            nc.gpsimd.memset(ones_row[:], 1.0)
            qtr_dram = dpool.tile([128, QH, S], BF16)   # roped Q^T scratch

            # ------- Phases A+B share an SBUF-resident x (16 MB) --------
            with tc.tile_pool(name="xres", bufs=1) as xrp:
                x_sb = xrp.tile([128, HT, S], BF16)

                # ---------------- Phase A: K/V projection + K rope ------
                # (streams x into x_sb for phase B to reuse)
                with (
                    tc.tile_pool(name="wkv", bufs=1) as wkvp,
                    tc.tile_pool(name="ropea", bufs=2) as rpa,
                    tc.tile_pool(name="psk", bufs=2, space="PSUM") as psk,
                    tc.tile_pool(name="psv", bufs=1, space="PSUM") as psv,
                ):
                    wk_sb = wkvp.tile([128, HT, KVROWS], BF16)
                    wv_sb = wkvp.tile([128, HT, KVROWS], BF16)
                    for q4 in range(4):          # seq quarters of 512
                        sl = slice(q4 * 512, (q4 + 1) * 512)
                        kps = psk.tile([128, KVH, 512], F32, name="kps")
                        vps = [psv.tile([128, KVROWS], F32, name=f"vps{st}")
                               for st in range(4)]
                        for h in range(HT):
                            if q4 == 0:
                                nc.sync.dma_start(wk_sb[:, h, :],
                                                  wkt[h * 128:(h + 1) * 128, :])
                                nc.sync.dma_start(wv_sb[:, h, :],
                                                  wvt[h * 128:(h + 1) * 128, :])
                            xa = x_sb[:, h, sl]
                            nc.sync.dma_start(xa, xtb[h * 128:(h + 1) * 128, sl])
                            for r in range(KVH):
                                nc.tensor.matmul(kps[:, r, :],
                                                 wk_sb[:, h, r * 128:(r + 1) * 128],
                                                 xa,
                                                 start=(h == 0), stop=(h == HT - 1))
                            for st in range(4):
                                nc.tensor.matmul(vps[st][:],
                                                 xa[:, st * 128:(st + 1) * 128],
                                                 wv_sb[:, h, :],
                                                 start=(h == 0), stop=(h == HT - 1))
                        # rope K -> ktr: rot = x*cc + shuffle(x*ss)
                        for r in range(KVH):
                            t1 = rpa.tile([128, 512], BF16, name="t1")
                            m0 = rpa.tile([128, 512], BF16, name="m0")
                            sw = rpa.tile([128, 512], BF16, name="sw")
                            nc.vector.tensor_tensor(t1[:], kps[:, r, :], cs_sb[:, sl], op=AX.mult)
                            nc.vector.tensor_tensor(m0[:], kps[:, r, :], sc_sb[:, sl], op=AX.mult)
                            nc.vector.stream_shuffle(sw[:], m0[:], mask=SWAP_ADJ)
                            nc.vector.tensor_tensor(ktr[:, r, sl], t1[:], sw[:], op=AX.add)
                        # evict V -> vb: vps[st][p, kv*128+d] -> vb[p, kv, q4*4+st, d]
                        for st in range(4):
                            nc.scalar.copy(
                                vb[:, :, q4 * 4 + st, :],
                                vps[st][:].rearrange("p (kv d) -> p kv d", kv=KVH))

                # ---------------- Phase B: Q projection + rope ----------
                # wq streamed in quarters (2 heads each); for a fixed
                # (head, h-tile) the stationary weight feeds all 4 seq
                # chunks back-to-back (no weight-reload bubble).
                with (
                    tc.tile_pool(name="wq", bufs=2) as wqp,
                    tc.tile_pool(name="ropeb", bufs=2) as rpb,
                    tc.tile_pool(name="psb", bufs=2, space="PSUM") as psb,
                ):
                    for rq in range(4):
                        wq_sb = wqp.tile([128, HT, 256], BF16, name="wq_sb")
                        for h in range(HT):
                            nc.sync.dma_start(
                                wq_sb[:, h, :],
                                wqt[h * 128:(h + 1) * 128, rq * 256:(rq + 1) * 256])
                        for r2 in range(2):
                            head = rq * 2 + r2
                            qps = psb.tile([128, 4, 512], F32, name="qps")
                            for h in range(HT):
                                for qc in range(4):
                                    nc.tensor.matmul(
                                        qps[:, qc, :],
                                        wq_sb[:, h, r2 * 128:(r2 + 1) * 128],
                                        x_sb[:, h, qc * 512:(qc + 1) * 512],
                                        start=(h == 0), stop=(h == HT - 1))
                            for qc in range(4):
                                sl = slice(qc * 512, (qc + 1) * 512)
                                t1 = rpb.tile([128, 512], BF16, name="t1")
                                m0 = rpb.tile([128, 512], BF16, name="m0")
                                sw = rpb.tile([128, 512], BF16, name="sw")
                                qs = rpb.tile([128, 512], BF16, name="qs")
                                nc.vector.tensor_tensor(t1[:], qps[:, qc, :], cs_sb[:, sl], op=AX.mult)
                                nc.vector.tensor_tensor(m0[:], qps[:, qc, :], sc_sb[:, sl], op=AX.mult)
                                nc.vector.stream_shuffle(sw[:], m0[:], mask=SWAP_ADJ)
                                nc.vector.tensor_tensor(qs[:], t1[:], sw[:], op=AX.add)
                                nc.sync.dma_start(qtr_dram[:, head, sl], qs[:])

            # ---------------- Phases C+D scope --------------------------
            with (
                tc.tile_pool(name="ctxp", bufs=1) as ctxp,
                tc.tile_pool(name="wo", bufs=1) as wop,
            ):
                ctx = ctxp.tile([128, QH, S], BF16)     # attention output (4 MB)
                wo_sb = wop.tile([128, QH, H], BF16)    # prefetched during C
                for h in range(QH):
                    nc.sync.dma_start(wo_sb[:, h, :], wot[h * 128:(h + 1) * 128, :])

                # ---------------- Phase C: attention --------------------
                def normalize(p_ctxps, p_dps, heads, qsl):
                    for i in range(2):
                        rf = smc.tile([1, QCH], F32, name="rf")
                        nc.vector.reciprocal_approx_fast(rf[:], p_dps[i][:])
                        rb = smc.tile([1, QCH], BF16, name="rb")
                        nc.vector.tensor_copy(rb[:], rf[:])
                        un = smc.tile([128, QCH], BF16, name="un")
                        nc.vector.tensor_copy(un[:], p_ctxps[i][:])
                        bps = pscs.tile([128, 2, QCH], F32, name="sp")
                        nc.tensor.matmul(bps[:, 0, :], ones_row[:], rb[:],
                                         start=True, stop=True)
                        nc.vector.tensor_tensor(
                            ctx[:, heads[i], qsl], un[:], bps[:, 0, :], op=AX.mult)

                with (
                    tc.tile_pool(name="qin", bufs=2) as qip,
                    tc.tile_pool(name="expp", bufs=6) as expp,
                    tc.tile_pool(name="sump", bufs=3) as sump,
                    tc.tile_pool(name="smallc", bufs=4) as smc,
                    tc.tile_pool(name="mkp", bufs=2) as mkp,
                    tc.tile_pool(name="pscs", bufs=2, space="PSUM") as pscs,
                    tc.tile_pool(name="pscx", bufs=2, space="PSUM") as pscx,
                    tc.tile_pool(name="pscd", bufs=2, space="PSUM") as pscd,
                ):
                    prev = None
                    for qq in range(NQQ):
                        qsl = slice(qq * QCH, (qq + 1) * QCH)
                        NKT = 4 * qq + 4 if causal else ST
                        if causal:
                            # diagonal tiles first, then strictly-lower kts
                            order = list(range(NKT - 4, NKT)) + list(range(NKT - 4))
                        else:
                            order = list(range(NKT))
                        qin = qip.tile([128, QH, QCH], BF16, name="qin")
                        for h in range(QH):
                            nc.sync.dma_start(qin[:, h, :], qtr_dram[:, h, qsl])
                        mk = None
                        if genmask:
                            mk = mkp.tile([128, ST, QCH], BF16, name="mk")
                            for kt in range(ST):
                                nc.sync.dma_start(
                                    mk[:, kt, :], maskt[kt * 128:(kt + 1) * 128, qsl])
                        for hp in range(QH // 2):
                            kvh = hp // (QH // KVH // 2)
                            heads = (2 * hp, 2 * hp + 1)
                            ctxps = [pscx.tile([128, QCH], F32, name="ctxps")
                                     for _ in range(2)]
                            dps = [pscd.tile([1, QCH], F32, name="dps")
                                   for _ in range(2)]
                            eps = [None] * NKT
                            esums = [None] * (NKT // 2)
                            for t in range(NKT + 5):
                                if t < NKT:
                                    kt = order[t]
                                    sp = pscs.tile([128, 2, QCH], F32, name="sp")
                                    for i in range(2):
                                        nc.tensor.matmul(
                                            sp[:, i, :],
                                            ktr[:, kvh, kt * 128:(kt + 1) * 128],
                                            qin[:, heads[i], :],
                                            start=True, stop=True)
                                        if genmask:
                                            nc.vector.tensor_tensor(
                                                sp[:, i, :], sp[:, i, :],
                                                mk[:, kt, :], op=AX.add)
                                    ep = expp.tile([128, 2, QCH], BF16, name="ep")
                                    nc.scalar.activation(ep[:], sp[:], ACTF.Exp)
                                    if causal and kt >= NKT - 4:
                                        d = kt - (NKT - 4)
                                        for i in range(2):
                                            nc.vector.tensor_tensor(
                                                ep[:, i, :], ep[:, i, :],
                                                diag_sb[:, d, :], op=AX.mult)
                                    eps[t] = ep
                                    if t % 2 == 1:
                                        es = sump.tile([128, 2, QCH], BF16, name="es")
                                        for i in range(2):
                                            nc.gpsimd.tensor_tensor(
                                                es[:, i, :], eps[t - 1][:, i, :],
                                                eps[t][:, i, :], op=AX.add)
                                        esums[t // 2] = es
                                if t == 1 and prev is not None:
                                    normalize(*prev)
                                    prev = None
                                ta = t - 3
                                if 0 <= ta < NKT:
                                    ep = eps[ta]
                                    kt = order[ta]
                                    for i in range(2):
                                        nc.tensor.matmul(
                                            ctxps[i][:], vb[:, kvh, kt, :], ep[:, i, :],
                                            start=(ta == 0), stop=(ta == NKT - 1))
                                td = t - 5
                                if 0 <= td < NKT and td % 2 == 1:
                                    pi = td // 2
                                    es = esums[pi]
                                    for i in range(2):
                                        nc.tensor.matmul(
                                            dps[i][:], ones_bf[:], es[:, i, :],
                                            start=(pi == 0), stop=(pi == NKT // 2 - 1))
                            prev = (ctxps, dps, heads, qsl)
                    if prev is not None:
                        normalize(*prev)

                if debug:
                    nc.sync.dma_start(dbg["k"][:], ktr[:].rearrange("p kv s -> p (kv s)"))
                    nc.sync.dma_start(dbg["v"][:], vb[:].rearrange("p kv st d -> p (kv st d)"))
                    nc.sync.dma_start(dbg["ctx"][:], ctx[:].rearrange("p h s -> p (h s)"))

                # ---------------- Phase D: output projection ------------
                with (
                    tc.tile_pool(name="ob", bufs=3) as obp,
                    tc.tile_pool(name="psd", bufs=2, space="PSUM") as psd,
                ):
                    for st in range(ST):
                        for half in range(2):
                            ops = psd.tile([128, 2048], F32, name="ops")
                            for h in range(QH):
                                for n in range(4):
                                    nc.tensor.matmul(
                                        ops[:, n * 512:(n + 1) * 512],
                                        ctx[:, h, st * 128:(st + 1) * 128],
                                        wo_sb[:, h, half * 2048 + n * 512:half * 2048 + (n + 1) * 512],
                                        start=(h == 0), stop=(h == QH - 1))
                            osb = obp.tile([128, 2048], F32, name="osb")
                            nc.scalar.copy(osb[:], ops[:])
                            nc.sync.dma_start(
                                out[st * 128:(st + 1) * 128,
                                    half * 2048:(half + 1) * 2048],
                                osb[:])

    nc.compile()
    nc.m = get_hw_module(nc.m)
    return nc


_NC_CACHE = {}


def _get_nc(mode: str, debug: bool = False):
    key = (mode, debug)
    if key not in _NC_CACHE:
        _NC_CACHE[key] = build_nc(mode, debug)
    return _NC_CACHE[key]


def _detect_mode(attention_mask):
    if not np.any(attention_mask):
        return "nomask"
    tril = np.tril(np.ones((S, S), dtype=bool))
    for b in range(attention_mask.shape[0]):
        m = attention_mask[b, 0]
        if not (np.all(m[tril] == 0.0) and np.all(m[~tril] < -1e30)):
            return "genmask"
    return "causal"


def _build_diag_pattern():
    # diag tile t: rows k (partition), cols q in [0, QCH);
    # unmasked (1.0) iff t*128 + k <= q
    k = np.arange(128)[:, None]
    q = np.arange(QCH)[None, :]
    tiles = [(t * 128 + k <= q).astype(np.float32) for t in range(4)]
    return np.concatenate(tiles, axis=1).astype(ml_dtypes.bfloat16)


def kernel(hidden_states, cos, sin, position_ids, attention_mask, Wq, Wk, Wv, Wo,
           _trace=False, _debug=False):
    hidden_states = np.asarray(hidden_states, np.float32)
    cos = np.asarray(cos, np.float32)
    sin = np.asarray(sin, np.float32)
    position_ids = np.asarray(position_ids)
    attention_mask = np.asarray(attention_mask, np.float32)
    Wq = np.asarray(Wq, np.float32)
    Wk = np.asarray(Wk, np.float32)
    Wv = np.asarray(Wv, np.float32)
    Wo = np.asarray(Wo, np.float32)

    mode = _detect_mode(attention_mask)
    nc = _get_nc(mode, _debug)

    scale = 1.0 / math.sqrt(HD)
    wqt_full = np.ascontiguousarray((Wq * scale).T).astype(ml_dtypes.bfloat16)
    wkt_full = np.ascontiguousarray(Wk.T).astype(ml_dtypes.bfloat16)
    wvt_full = np.ascontiguousarray(Wv.T).astype(ml_dtypes.bfloat16)
    wot_full = np.ascontiguousarray(Wo.T).astype(ml_dtypes.bfloat16)

    pos = np.asarray(position_ids, np.int64)
    diag = _build_diag_pattern() if mode == "causal" else None
    per_batch = {}
    for b in range(B):
        xtb = np.ascontiguousarray(hidden_states[b].T).astype(ml_dtypes.bfloat16)
        cg = cos[pos[b]]                                     # [2048, 64]
        sg = sin[pos[b]]
        cs_b = np.repeat(cg.T, 2, axis=0).astype(ml_dtypes.bfloat16)   # cc
        sc_b = np.empty((HD, S), np.float32)                           # ss
        sc_b[0::2] = sg.T
        sc_b[1::2] = -sg.T
        sc_b = sc_b.astype(ml_dtypes.bfloat16)
        mt_b = None
        if mode == "genmask":
            mt_b = np.ascontiguousarray(attention_mask[b, 0].T).astype(ml_dtypes.bfloat16)
        per_batch[b] = (xtb, cs_b, sc_b, mt_b)

    in_maps = []
    for c in range(N_CORES):
        b, tp = c // TP, c % TP
        xtb, cs_b, sc_b, mt_b = per_batch[b]
        m = {
            "xtb": xtb,
            "wqt": np.ascontiguousarray(wqt_full[:, tp * QROWS:(tp + 1) * QROWS]),
            "wkt": np.ascontiguousarray(wkt_full[:, tp * KVROWS:(tp + 1) * KVROWS]),
            "wvt": np.ascontiguousarray(wvt_full[:, tp * KVROWS:(tp + 1) * KVROWS]),
            "wot": np.ascontiguousarray(wot_full[tp * QROWS:(tp + 1) * QROWS, :]),
            "cs": cs_b,
            "sc": sc_b,
        }
        if mode == "causal":
            m["diagm"] = diag
        if mode == "genmask":
            m["maskt"] = mt_b
        in_maps.append(m)

    res = bass_utils.run_bass_kernel_spmd(
        nc, in_maps, core_ids=list(range(N_CORES)), trace=_trace)

    out = np.zeros((B, S, H), np.float32)
    for c in range(N_CORES):
        out[c // TP] += res.results[c]["out"]
    if _trace:
        kernel._last_results = res
    return out


# revision 25
# speedup vs baseline: 1.3983x; 1.0118x over previous
"""GQA attention layer (B=2, S=2048, H=4096, 32 Q heads / 8 KV heads, HD=128)
on 8 trn2 NeuronCores.

Sharding: 2D = data-parallel over batch (2) x tensor-parallel over heads (4).
Core c -> (batch = c // 4, tp = c % 4): 8 Q heads, 2 KV heads, full sequence.
Wq/Wk/Wv split along output rows, Wo along input cols (Megatron TP); the
4 per-batch partial outputs are summed on the host (the TP unshard step).

All matmuls run in bf16 (1 cycle/col).  x^T is streamed once into SBUF
(16 MB) during phase A and stays resident through phase B, so the Q
projection can reuse each stationary weight tile across 4 moving seq
chunks (the PE runs same-stationary matmuls back-to-back without a
weight-reload bubble).  Per-core phases:
  A: K/V projections (streams x into SBUF), RoPE on K     -> ktr, vb (SBUF)
  B: Q projection + RoPE                                  -> qtr (DRAM)
  C: attention per (q-chunk, head-pair), see below        -> ctx (SBUF, bf16)
  D: out = ctx^T x Wo^T (bf16, fp32 accum)                -> out (DRAM, fp32)

Phase C processes heads in pairs sharing the kv head: the two scores (and
AV) matmuls per kt share the stationary operand; one exp (ACT) per kt
covers both heads.  The denominator uses kt-pair sums computed on the
otherwise-idle GpSimd engine, halving the ones-matmul count.  AV lags exp
by 3 slots and the denominator by 5 so the PE never waits on ACT/GpSimd.

Modes:
  causal : skip kt tiles above the diagonal (kt > 4*qq+3); the 4 diagonal
           tiles are masked multiplicatively after exp with a precomputed
           0/1 pattern (the same pattern for every q-chunk), and are
           processed first so the DVE masking stays off the steady-state
           exp->AV path.
  nomask : all 16 kt tiles, no masking.
  genmask: all 16 kt tiles, additive mask tiles streamed from DRAM
           (insurance path for non-causal non-zero masks).

RoPE runs in the natural interleaved head layout: pair (x[2i], x[2i+1])
sits at adjacent partitions, the partner is fetched with a swap-adjacent
stream_shuffle, and the sign/cos/sin tables are pre-interleaved on the host:
  rot = x * cc + shuffle(x * ss),  cc[2i]=cc[2i+1]=cos_i,
  ss[2i]=+sin_i, ss[2i+1]=-sin_i.
"""

import math

import numpy as np
import ml_dtypes

import concourse.bass as bass
import concourse.mybir as mybir
import concourse.tile as tile
from concourse import bacc
from concourse import bass_utils
from concourse.bass_interp import get_hw_module

B, S, H, NH, NKV, HD = 2, 2048, 4096, 32, 8, 128
TP = 4  # head-parallel cores per batch
N_CORES = 8
QH = NH // TP          # 8 q heads per core
KVH = NKV // TP        # 2 kv heads per core
QROWS = QH * HD        # 1024
KVROWS = KVH * HD      # 256
HT = H // 128          # 32 h (contraction) tiles
ST = S // 128          # 16 seq tiles
QCH = 512              # q-chunk width in phase C
NQQ = S // QCH
F32 = mybir.dt.float32
BF16 = mybir.dt.bfloat16
AX = mybir.AluOpType
ACTF = mybir.ActivationFunctionType
SWAP_ADJ = [i ^ 1 for i in range(32)]


def build_nc(mode: str, debug: bool = False):
    causal = mode == "causal"
    genmask = mode == "genmask"

    nc = bacc.Bacc("TRN2", target_bir_lowering=False, debug=False, num_devices=N_CORES)
    xtb = nc.dram_tensor("xtb", [H, S], BF16, kind="ExternalInput").ap()
    wqt = nc.dram_tensor("wqt", [H, QROWS], BF16, kind="ExternalInput").ap()
    wkt = nc.dram_tensor("wkt", [H, KVROWS], BF16, kind="ExternalInput").ap()
    wvt = nc.dram_tensor("wvt", [H, KVROWS], BF16, kind="ExternalInput").ap()
    wot = nc.dram_tensor("wot", [QROWS, H], BF16, kind="ExternalInput").ap()
    cs = nc.dram_tensor("cs", [128, S], BF16, kind="ExternalInput").ap()
    sc = nc.dram_tensor("sc", [128, S], BF16, kind="ExternalInput").ap()
    diagm = None
    maskt = None
    if causal:
        diagm = nc.dram_tensor("diagm", [128, 4 * QCH], BF16, kind="ExternalInput").ap()
    if genmask:
        maskt = nc.dram_tensor("maskt", [S, S], BF16, kind="ExternalInput").ap()
    out = nc.dram_tensor("out", [S, H], F32, kind="ExternalOutput").ap()
    dbg = {}
    if debug:
        dbg["k"] = nc.dram_tensor("dbg_k", [128, KVH * S], BF16, kind="ExternalOutput").ap()
        dbg["v"] = nc.dram_tensor("dbg_v", [128, KVH * ST * HD], BF16, kind="ExternalOutput").ap()
        dbg["ctx"] = nc.dram_tensor("dbg_ctx", [128, QH * S], BF16, kind="ExternalOutput").ap()

    with tile.TileContext(nc) as tc:
        with (
            tc.tile_pool(name="persist", bufs=1) as pp,
            tc.tile_pool(name="dram", bufs=1, space="DRAM") as dpool,
        ):
            cs_sb = pp.tile([128, S], BF16)
            sc_sb = pp.tile([128, S], BF16)
            ktr = pp.tile([128, KVH, S], BF16)          # roped K^T (1 MB)
            vb = pp.tile([128, KVH, ST, HD], BF16)      # V [seq, hd] tiles (1 MB)
            ones_bf = pp.tile([128, 1], BF16)
            ones_row = pp.tile([1, 128], BF16)
            diag_sb = None
            if causal:
                diag_sb = pp.tile([128, 4, QCH], BF16)
                nc.sync.dma_start(diag_sb[:],
                                  diagm[:].rearrange("p (t q) -> p t q", t=4))
            nc.sync.dma_start(cs_sb[:], cs[:])
            nc.sync.dma_start(sc_sb[:], sc[:])
            nc.gpsimd.memset(ones_bf[:], 1.0)
            nc.gpsimd.memset(ones_row[:], 1.0)
            qtr_dram = dpool.tile([128, QH, S], BF16)   # roped Q^T scratch

            # ------- Phases A+B share an SBUF-resident x (16 MB) --------
            with tc.tile_pool(name="xres", bufs=1) as xrp:
                x_sb = xrp.tile([128, HT, S], BF16)

                # ---------------- Phase A: K/V projection + K rope ------
                # (streams x into x_sb for phase B to reuse)
                with (
                    tc.tile_pool(name="wkv", bufs=1) as wkvp,
                    tc.tile_pool(name="ropea", bufs=2) as rpa,
                    tc.tile_pool(name="psk", bufs=2, space="PSUM") as psk,
                    tc.tile_pool(name="psv", bufs=1, space="PSUM") as psv,
                ):
                    wk_sb = wkvp.tile([128, HT, KVROWS], BF16)
                    wv_sb = wkvp.tile([128, HT, KVROWS], BF16)
                    for q4 in range(4):          # seq quarters of 512
                        sl = slice(q4 * 512, (q4 + 1) * 512)
                        kps = psk.tile([128, KVH, 512], F32, name="kps")
                        vps = [psv.tile([128, KVROWS], F32, name=f"vps{st}")
                               for st in range(4)]
                        for h in range(HT):
                            if q4 == 0:
                                nc.sync.dma_start(wk_sb[:, h, :],
                                                  wkt[h * 128:(h + 1) * 128, :])
                                nc.sync.dma_start(wv_sb[:, h, :],
                                                  wvt[h * 128:(h + 1) * 128, :])
                            xa = x_sb[:, h, sl]
                            nc.sync.dma_start(xa, xtb[h * 128:(h + 1) * 128, sl])
                            for r in range(KVH):
                                nc.tensor.matmul(kps[:, r, :],
                                                 wk_sb[:, h, r * 128:(r + 1) * 128],
                                                 xa,
                                                 start=(h == 0), stop=(h == HT - 1))
                            for st in range(4):
                                nc.tensor.matmul(vps[st][:],
                                                 xa[:, st * 128:(st + 1) * 128],
                                                 wv_sb[:, h, :],
                                                 start=(h == 0), stop=(h == HT - 1))
                        # rope K -> ktr: rot = x*cc + shuffle(x*ss)
                        for r in range(KVH):
                            t1 = rpa.tile([128, 512], BF16, name="t1")
                            m0 = rpa.tile([128, 512], BF16, name="m0")
                            sw = rpa.tile([128, 512], BF16, name="sw")
                            nc.vector.tensor_tensor(t1[:], kps[:, r, :], cs_sb[:, sl], op=AX.mult)
                            nc.vector.tensor_tensor(m0[:], kps[:, r, :], sc_sb[:, sl], op=AX.mult)
                            nc.vector.stream_shuffle(sw[:], m0[:], mask=SWAP_ADJ)
                            nc.vector.tensor_tensor(ktr[:, r, sl], t1[:], sw[:], op=AX.add)
                        # evict V -> vb: vps[st][p, kv*128+d] -> vb[p, kv, q4*4+st, d]
                        for st in range(4):
                            nc.scalar.copy(
                                vb[:, :, q4 * 4 + st, :],
                                vps[st][:].rearrange("p (kv d) -> p kv d", kv=KVH))

                # ---------------- Phase B: Q projection + rope ----------
                # wq streamed in quarters (2 heads each); for a fixed
                # (head, h-tile) the stationary weight feeds all 4 seq
                # chunks back-to-back (no weight-reload bubble).
                with (
                    tc.tile_pool(name="wq", bufs=2) as wqp,
                    tc.tile_pool(name="ropeb", bufs=2) as rpb,
                    tc.tile_pool(name="psb", bufs=2, space="PSUM") as psb,
                ):
                    for rq in range(4):
                        wq_sb = wqp.tile([128, HT, 256], BF16, name="wq_sb")
                        for h in range(HT):
                            nc.sync.dma_start(
                                wq_sb[:, h, :],
                                wqt[h * 128:(h + 1) * 128, rq * 256:(rq + 1) * 256])
                        for r2 in range(2):
                            head = rq * 2 + r2
                            qps = psb.tile([128, 4, 512], F32, name="qps")
                            for h in range(HT):
                                for qc in range(4):
                                    nc.tensor.matmul(
                                        qps[:, qc, :],
                                        wq_sb[:, h, r2 * 128:(r2 + 1) * 128],
                                        x_sb[:, h, qc * 512:(qc + 1) * 512],
                                        start=(h == 0), stop=(h == HT - 1))
                            for qc in range(4):
                                sl = slice(qc * 512, (qc + 1) * 512)
                                t1 = rpb.tile([128, 512], BF16, name="t1")
                                m0 = rpb.tile([128, 512], BF16, name="m0")
                                sw = rpb.tile([128, 512], BF16, name="sw")
                                qs = rpb.tile([128, 512], BF16, name="qs")
                                nc.vector.tensor_tensor(t1[:], qps[:, qc, :], cs_sb[:, sl], op=AX.mult)
                                nc.vector.tensor_tensor(m0[:], qps[:, qc, :], sc_sb[:, sl], op=AX.mult)
                                nc.vector.stream_shuffle(sw[:], m0[:], mask=SWAP_ADJ)
                                nc.vector.tensor_tensor(qs[:], t1[:], sw[:], op=AX.add)
                                nc.sync.dma_start(qtr_dram[:, head, sl], qs[:])

            # ---------------- Phases C+D scope --------------------------
            with (
                tc.tile_pool(name="ctxp", bufs=1) as ctxp,
                tc.tile_pool(name="wo", bufs=1) as wop,
            ):
                ctx = ctxp.tile([128, QH, S], BF16)     # attention output (4 MB)
                wo_sb = wop.tile([128, QH, H], BF16)    # prefetched during C

                # ---------------- Phase C: attention --------------------
                def normalize(p_ctxps, p_dps, heads, qsl):
                    for i in range(2):
                        rf = smc.tile([1, QCH], F32, name="rf")
                        nc.vector.reciprocal_approx_fast(rf[:], p_dps[i][:])
                        rb = smc.tile([1, QCH], BF16, name="rb")
                        nc.vector.tensor_copy(rb[:], rf[:])
                        un = smc.tile([128, QCH], BF16, name="un")
                        nc.vector.tensor_copy(un[:], p_ctxps[i][:])
                        bps = pscs.tile([128, 2, QCH], F32, name="sp")
                        nc.tensor.matmul(bps[:, 0, :], ones_row[:], rb[:],
                                         start=True, stop=True)
                        nc.vector.tensor_tensor(
                            ctx[:, heads[i], qsl], un[:], bps[:, 0, :], op=AX.mult)

                with (
                    tc.tile_pool(name="qin", bufs=2) as qip,
                    tc.tile_pool(name="expp", bufs=6) as expp,
                    tc.tile_pool(name="sump", bufs=3) as sump,
                    tc.tile_pool(name="smallc", bufs=4) as smc,
                    tc.tile_pool(name="mkp", bufs=2) as mkp,
                    tc.tile_pool(name="pscs", bufs=2, space="PSUM") as pscs,
                    tc.tile_pool(name="pscx", bufs=2, space="PSUM") as pscx,
                    tc.tile_pool(name="pscd", bufs=2, space="PSUM") as pscd,
                ):
                    def load_qin(qq):
                        qin = qip.tile([128, QH, QCH], BF16, name="qin")
                        for h in range(QH):
                            nc.sync.dma_start(
                                qin[:, h, :],
                                qtr_dram[:, h, qq * QCH:(qq + 1) * QCH])
                        return qin

                    qin_next = load_qin(0)
                    # wo prefetch queued after the first q tiles so phase C
                    # doesn't wait on it
                    for h in range(QH):
                        nc.sync.dma_start(wo_sb[:, h, :],
                                          wot[h * 128:(h + 1) * 128, :])
                    prev = None
                    for qq in range(NQQ):
                        qsl = slice(qq * QCH, (qq + 1) * QCH)
                        NKT = 4 * qq + 4 if causal else ST
                        if causal:
                            # diagonal tiles first, then strictly-lower kts
                            order = list(range(NKT - 4, NKT)) + list(range(NKT - 4))
                        else:
                            order = list(range(NKT))
                        qin = qin_next
                        mk = None
                        if genmask:
                            mk = mkp.tile([128, ST, QCH], BF16, name="mk")
                            for kt in range(ST):
                                nc.sync.dma_start(
                                    mk[:, kt, :], maskt[kt * 128:(kt + 1) * 128, qsl])
                        for hp in range(QH // 2):
                            if hp == QH // 2 - 1 and qq + 1 < NQQ:
                                qin_next = load_qin(qq + 1)
                            kvh = hp // (QH // KVH // 2)
                            heads = (2 * hp, 2 * hp + 1)
                            ctxps = [pscx.tile([128, QCH], F32, name="ctxps")
                                     for _ in range(2)]
                            dps = [pscd.tile([1, QCH], F32, name="dps")
                                   for _ in range(2)]
                            eps = [None] * NKT
                            esums = [None] * (NKT // 2)
                            for t in range(NKT + 5):
                                if t < NKT:
                                    kt = order[t]
                                    sp = pscs.tile([128, 2, QCH], F32, name="sp")
                                    for i in range(2):
                                        nc.tensor.matmul(
                                            sp[:, i, :],
                                            ktr[:, kvh, kt * 128:(kt + 1) * 128],
                                            qin[:, heads[i], :],
                                            start=True, stop=True)
                                        if genmask:
                                            nc.vector.tensor_tensor(
                                                sp[:, i, :], sp[:, i, :],
                                                mk[:, kt, :], op=AX.add)
                                    ep = expp.tile([128, 2, QCH], BF16, name="ep")
                                    nc.scalar.activation(ep[:], sp[:], ACTF.Exp)
                                    if causal and kt >= NKT - 4:
                                        d = kt - (NKT - 4)
                                        for i in range(2):
                                            nc.vector.tensor_tensor(
                                                ep[:, i, :], ep[:, i, :],
                                                diag_sb[:, d, :], op=AX.mult)
                                    eps[t] = ep
                                    if t % 2 == 1:
                                        es = sump.tile([128, 2, QCH], BF16, name="es")
                                        for i in range(2):
                                            nc.gpsimd.tensor_tensor(
                                                es[:, i, :], eps[t - 1][:, i, :],
                                                eps[t][:, i, :], op=AX.add)
                                        esums[t // 2] = es
                                if t == 3 and prev is not None:
                                    # deferred one group so the DVE queue
                                    # serves the diagonal-mask multiplies
                                    # (which gate AV) first
                                    normalize(*prev)
                                    prev = None
                                ta = t - 3
                                if 0 <= ta < NKT:
                                    ep = eps[ta]
                                    kt = order[ta]
                                    for i in range(2):
                                        nc.tensor.matmul(
                                            ctxps[i][:], vb[:, kvh, kt, :], ep[:, i, :],
                                            start=(ta == 0), stop=(ta == NKT - 1))
                                td = t - 5
                                if 0 <= td < NKT and td % 2 == 1:
                                    pi = td // 2
                                    es = esums[pi]
                                    for i in range(2):
                                        nc.tensor.matmul(
                                            dps[i][:], ones_bf[:], es[:, i, :],
                                            start=(pi == 0), stop=(pi == NKT // 2 - 1))
                            prev = (ctxps, dps, heads, qsl)
                    if prev is not None:
                        normalize(*prev)

                if debug:
                    nc.sync.dma_start(dbg["k"][:], ktr[:].rearrange("p kv s -> p (kv s)"))
                    nc.sync.dma_start(dbg["v"][:], vb[:].rearrange("p kv st d -> p (kv st d)"))
                    nc.sync.dma_start(dbg["ctx"][:], ctx[:].rearrange("p h s -> p (h s)"))

                # ---------------- Phase D: output projection ------------
                with (
                    tc.tile_pool(name="ob", bufs=3) as obp,
                    tc.tile_pool(name="psd", bufs=2, space="PSUM") as psd,
                ):
                    for st in range(ST):
                        for half in range(2):
                            ops = psd.tile([128, 2048], F32, name="ops")
                            for h in range(QH):
                                for n in range(4):
                                    nc.tensor.matmul(
                                        ops[:, n * 512:(n + 1) * 512],
                                        ctx[:, h, st * 128:(st + 1) * 128],
                                        wo_sb[:, h, half * 2048 + n * 512:half * 2048 + (n + 1) * 512],
                                        start=(h == 0), stop=(h == QH - 1))
                            osb = obp.tile([128, 2048], F32, name="osb")
                            nc.scalar.copy(osb[:], ops[:])
                            nc.sync.dma_start(
                                out[st * 128:(st + 1) * 128,
                                    half * 2048:(half + 1) * 2048],
                                osb[:])

    nc.compile()
    nc.m = get_hw_module(nc.m)
    return nc


_NC_CACHE = {}


def _get_nc(mode: str, debug: bool = False):
    key = (mode, debug)
    if key not in _NC_CACHE:
        _NC_CACHE[key] = build_nc(mode, debug)
    return _NC_CACHE[key]


def _detect_mode(attention_mask):
    if not np.any(attention_mask):
        return "nomask"
    tril = np.tril(np.ones((S, S), dtype=bool))
    for b in range(attention_mask.shape[0]):
        m = attention_mask[b, 0]
        if not (np.all(m[tril] == 0.0) and np.all(m[~tril] < -1e30)):
            return "genmask"
    return "causal"


def _build_diag_pattern():
    # diag tile t: rows k (partition), cols q in [0, QCH);
    # unmasked (1.0) iff t*128 + k <= q
    k = np.arange(128)[:, None]
    q = np.arange(QCH)[None, :]
    tiles = [(t * 128 + k <= q).astype(np.float32) for t in range(4)]
    return np.concatenate(tiles, axis=1).astype(ml_dtypes.bfloat16)


def kernel(hidden_states, cos, sin, position_ids, attention_mask, Wq, Wk, Wv, Wo,
           _trace=False, _debug=False):
    hidden_states = np.asarray(hidden_states, np.float32)
    cos = np.asarray(cos, np.float32)
    sin = np.asarray(sin, np.float32)
    position_ids = np.asarray(position_ids)
    attention_mask = np.asarray(attention_mask, np.float32)
    Wq = np.asarray(Wq, np.float32)
    Wk = np.asarray(Wk, np.float32)
    Wv = np.asarray(Wv, np.float32)
    Wo = np.asarray(Wo, np.float32)

    mode = _detect_mode(attention_mask)
    nc = _get_nc(mode, _debug)

    scale = 1.0 / math.sqrt(HD)
    wqt_full = np.ascontiguousarray((Wq * scale).T).astype(ml_dtypes.bfloat16)
    wkt_full = np.ascontiguousarray(Wk.T).astype(ml_dtypes.bfloat16)
    wvt_full = np.ascontiguousarray(Wv.T).astype(ml_dtypes.bfloat16)
    wot_full = np.ascontiguousarray(Wo.T).astype(ml_dtypes.bfloat16)

    pos = np.asarray(position_ids, np.int64)
    diag = _build_diag_pattern() if mode == "causal" else None
    per_batch = {}
    for b in range(B):
        xtb = np.ascontiguousarray(hidden_states[b].T).astype(ml_dtypes.bfloat16)
        cg = cos[pos[b]]                                     # [2048, 64]
        sg = sin[pos[b]]
        cs_b = np.repeat(cg.T, 2, axis=0).astype(ml_dtypes.bfloat16)   # cc
        sc_b = np.empty((HD, S), np.float32)                           # ss
        sc_b[0::2] = sg.T
        sc_b[1::2] = -sg.T
        sc_b = sc_b.astype(ml_dtypes.bfloat16)
        mt_b = None
        if mode == "genmask":
            mt_b = np.ascontiguousarray(attention_mask[b, 0].T).astype(ml_dtypes.bfloat16)
        per_batch[b] = (xtb, cs_b, sc_b, mt_b)

    in_maps = []
    for c in range(N_CORES):
        b, tp = c // TP, c % TP
        xtb, cs_b, sc_b, mt_b = per_batch[b]
        m = {
            "xtb": xtb,
            "wqt": np.ascontiguousarray(wqt_full[:, tp * QROWS:(tp + 1) * QROWS]),
            "wkt": np.ascontiguousarray(wkt_full[:, tp * KVROWS:(tp + 1) * KVROWS]),
            "wvt": np.ascontiguousarray(wvt_full[:, tp * KVROWS:(tp + 1) * KVROWS]),
            "wot": np.ascontiguousarray(wot_full[tp * QROWS:(tp + 1) * QROWS, :]),
            "cs": cs_b,
            "sc": sc_b,
        }
        if mode == "causal":
            m["diagm"] = diag
        if mode == "genmask":
            m["maskt"] = mt_b
        in_maps.append(m)

    res = bass_utils.run_bass_kernel_spmd(
        nc, in_maps, core_ids=list(range(N_CORES)), trace=_trace)

    out = np.zeros((B, S, H), np.float32)
    for c in range(N_CORES):
        out[c // TP] += res.results[c]["out"]
    if _trace:
        kernel._last_results = res
    return out
